# revision 15
# baseline (speedup 1.0000x reference)
# RWKV WKV log-space recurrence on 8 TRN2 NeuronCores.
#
# Reference recurrence (per (b, d), sequential over t, log-space):
#   lap' = logaddexp(lap - w, k + ln(relu(v)+eps))
#   lam' = logaddexp(lam - w, k + ln(relu(-v)+eps))
#   lb'  = logaddexp(lb - w,  k)
#   wkv  = exp(lae(u+k+ln v_p, lap) - lae(u+k, lb)) - exp(lae(u+k+ln v_m, lam) - lae(u+k, lb))
#
# Because w >= 0.05 > 0 the recurrence is a stable linear scan in exp space:
#   Ap' = e^-w Ap + e^k (relu(v)+eps),  Am' = e^-w Am + e^k (relu(-v)+eps),
#   B'  = e^-w B  + e^k,   wkv_t = (e^u e^k v + Ap - Am) / (e^u e^k + B).
# Scans are linear, so we scan sum/difference combinations directly:
#   Splus  : b_t = 2 eps e^k + 2 relu(+e^k v),  init 2 Ap0   -> lap = ln(.5 Splus)
#   Sminus : b_t = 2 eps e^k + 2 relu(-e^k v),  init 2 Am0   -> lam = ln(.5 Sminus)
#   Adif   : b_t = e^k v,                       init Ap0-Am0 -> wkv numerator
#   Bb     : b_t = e^k,                         init B0      -> lb, denominator
# (|x|+x = 2 relu(x) removes the Abs; Splus/Sminus replace Asum+-Adif.)
# Each scan is one DVE TensorTensorScanArith: state = d*state + b_t, chained
# across time chunks via the chunk's column-0 carry copy.
#
# Sharding: one batch element per core (B=8). Layout: channels on the 128
# partitions (8 groups of 128), time chunks of TC along the free dim. All
# DRAM I/O is channel-major ([D, T]); the host transposes in kernel() so
# every DMA descriptor is a contiguous 4KB row at full HBM rate.
import numpy as np
from contextlib import ExitStack

import concourse.bacc as bacc
import concourse.bass as bass
import concourse.hw_specs as hw_specs
import concourse.mybir as mybir
import concourse.tile as tile
from concourse.bass_utils import run_bass_kernel_spmd

F32 = mybir.dt.float32
AF = mybir.ActivationFunctionType
ALU = mybir.AluOpType

B, T, D = 8, 2048, 1024
P = 128
EPS = 1e-4

_ACT_TABLES_PATCHED = False


def _patch_act_tables():
    """Make the act-table-load pass always pick `natural_log_exp_and_others`
    (contains Exp+Ln+Relu) instead of thrashing between the exp-only and
    ln-only sets. We strip our functions from every set that precedes it in
    the list; positions (= act_func_set_ids) are unchanged, so walrus's
    remap stays valid."""
    global _ACT_TABLES_PATCHED
    if _ACT_TABLES_PATCHED:
        return
    _ACT_TABLES_PATCHED = True
    orig = hw_specs.get_activation_tables
    ours = {AF.Exp, AF.Ln, AF.Relu, AF.Abs}

    def patched(module_arch):
        tabs = orig(module_arch)
        out = {}
        seen_combined = False
        for name, s in tabs.items():
            if name == "natural_log_exp_and_others":
                seen_combined = True
                out[name] = s
            elif not seen_combined:
                out[name] = s - ours
            else:
                out[name] = s
        return out

    patched.__wrapped__ = orig
    hw_specs.get_activation_tables = patched
    bacc.get_activation_tables = patched


def build_nc(T=T, D=D, TC=2048, n_cores=8):
    """Per-core Bass graph (same NEFF on all cores). I/O is channel-major."""
    _patch_act_tables()
    G = D // P
    CI = T // TC
    nc = bacc.Bacc("TRN2", target_bir_lowering=False, debug=False,
                   num_devices=n_cores)
    k_d = nc.dram_tensor("k", [D, T], F32, kind="ExternalInput").ap()
    v_d = nc.dram_tensor("v", [D, T], F32, kind="ExternalInput").ap()
    w_d = nc.dram_tensor("w", [D], F32, kind="ExternalInput").ap()
    u_d = nc.dram_tensor("u", [D], F32, kind="ExternalInput").ap()
    st_d = nc.dram_tensor("state", [3, 1, D], F32, kind="ExternalInput").ap()
    wkv_d = nc.dram_tensor("wkv", [D, T], F32, kind="ExternalOutput").ap()
    sto_d = nc.dram_tensor("state_out", [3, D, T + 1], F32,
                           kind="ExternalOutput").ap()

    with tile.TileContext(nc) as tc, ExitStack() as ctx:
        const = ctx.enter_context(tc.tile_pool(name="const", bufs=2))
        io = ctx.enter_context(tc.tile_pool(name="io", bufs=2))     # kt, vt
        hot = ctx.enter_context(tc.tile_pool(name="hot", bufs=2))   # ek, ekv
        work = ctx.enter_context(tc.tile_pool(name="work", bufs=1))
        nump = ctx.enter_context(tc.tile_pool(name="nump", bufs=2))  # num/wkv
        scanp = ctx.enter_context(tc.tile_pool(name="scan", bufs=2))

        # t=0 column of the trajectory is the raw input state (12KB, once).
        with nc.allow_non_contiguous_dma(reason="12KB one-time init column"):
            nc.sync.dma_start(out=sto_d[:, :, 0:1],
                              in_=st_d[:, 0, :].unsqueeze(2))

        def make_emit_lns(c0):
            def emit_lns(t0, splus, smin, bb):
                nc.scalar.activation(splus[:, 1:TC + 1], splus[:, 1:TC + 1],
                                     AF.Ln)
                nc.sync.dma_start(out=sto_d[0, c0:c0 + P, t0 + 1:t0 + TC + 1],
                                  in_=splus[:, 1:TC + 1])
                nc.scalar.activation(smin[:, 1:TC + 1], smin[:, 1:TC + 1],
                                     AF.Ln)
                nc.sync.dma_start(out=sto_d[1, c0:c0 + P, t0 + 1:t0 + TC + 1],
                                  in_=smin[:, 1:TC + 1])
                nc.scalar.activation(bb[:, 1:TC + 1], bb[:, 1:TC + 1], AF.Ln)
                nc.sync.dma_start(out=sto_d[2, c0:c0 + P, t0 + 1:t0 + TC + 1],
                                  in_=bb[:, 1:TC + 1])
            return emit_lns

        for g in range(G):
            c0 = g * P
            emit_lns = make_emit_lns(c0)
            pending = []
            wg = const.tile([P, 1], F32, tag="wg")
            nc.sync.dma_start(out=wg[:], in_=w_d[c0:c0 + P].unsqueeze(1))
            ug = const.tile([P, 1], F32, tag="ug")
            nc.sync.dma_start(out=ug[:], in_=u_d[c0:c0 + P].unsqueeze(1))
            stg = const.tile([P, 3], F32, tag="stg")
            nc.sync.dma_start(out=stg[:], in_=st_d[:, 0, c0:c0 + P].transpose([1, 0]))

            dg = const.tile([P, 1], F32, tag="dg")
            nc.scalar.activation(dg[:], wg[:], AF.Exp, scale=-1.0)
            eug = const.tile([P, 1], F32, tag="eug")
            nc.scalar.activation(eug[:], ug[:], AF.Exp)
            est = const.tile([P, 3], F32, tag="est")
            nc.scalar.activation(est[:], stg[:], AF.Exp)
            i_plus = est[:, 0:1]
            i_minus = est[:, 1:2]
            i_b = est[:, 2:3]

            prev = None
            for ci in range(CI):
                t0 = ci * TC
                kt = io.tile([P, TC], F32, tag="kt")
                nc.sync.dma_start(out=kt[:], in_=k_d[c0:c0 + P, t0:t0 + TC])
                vt = io.tile([P, TC], F32, tag="vt")
                nc.sync.dma_start(out=vt[:], in_=v_d[c0:c0 + P, t0:t0 + TC])

                ek = hot.tile([P, TC], F32, tag="ek")
                nc.scalar.activation(ek[:], kt[:], AF.Exp)
                ekv = hot.tile([P, TC], F32, tag="ekv")
                nc.gpsimd.tensor_mul(ekv[:], ek[:], vt[:])
                rp = work.tile([P, TC], F32, tag="rp")
                nc.scalar.activation(rp[:], ekv[:], AF.Relu, scale=1.0)
                rm = work.tile([P, TC], F32, tag="rm")
                nc.scalar.activation(rm[:], ekv[:], AF.Relu, scale=-1.0)
                # eek = 2 eps e^k, euk = e^u e^k (ACT Copy with scale)
                eek = work.tile([P, TC], F32, tag="eek")
                nc.scalar.activation(eek[:], ek[:], AF.Copy, scale=1.0 * EPS)
                euk = work.tile([P, TC], F32, tag="euk")
                nc.scalar.activation(euk[:], ek[:], AF.Copy, scale=eug[:])
                sp_in = work.tile([P, TC], F32, tag="sp_in")
                nc.gpsimd.tensor_add(sp_in[:], eek[:], rp[:])
                sm_in = work.tile([P, TC], F32, tag="sm_in")
                nc.gpsimd.tensor_add(sm_in[:], eek[:], rm[:])

                splus = scanp.tile([P, TC + 1], F32, tag="splus")
                smin = scanp.tile([P, TC + 1], F32, tag="smin")
                bb = scanp.tile([P, TC + 1], F32, tag="bb")
                # column 0 = carry in (raw linear value, copied before the
                # previous chunk's in-place Ln clobbers it)
                if prev is None:
                    nc.vector.tensor_copy(splus[:, 0:1], i_plus)
                    nc.vector.tensor_copy(smin[:, 0:1], i_minus)
                    nc.vector.tensor_copy(bb[:, 0:1], i_b)
                else:
                    nc.vector.tensor_copy(splus[:, 0:1], prev[0][:, TC:TC + 1])
                    nc.vector.tensor_copy(smin[:, 0:1], prev[1][:, TC:TC + 1])
                    nc.vector.tensor_copy(bb[:, 0:1], prev[2][:, TC:TC + 1])
                dbc = dg[:].broadcast_to([P, TC])
                nc.vector.tensor_tensor_scan(
                    splus[:, 1:TC + 1], dbc, sp_in[:], splus[:, 0:1],
                    ALU.mult, ALU.add)
                nc.vector.tensor_tensor_scan(
                    smin[:, 1:TC + 1], dbc, sm_in[:], smin[:, 0:1],
                    ALU.mult, ALU.add)
                nc.vector.tensor_tensor_scan(
                    bb[:, 1:TC + 1], dbc, ek[:], bb[:, 0:1],
                    ALU.mult, ALU.add)

                # Adif_prev = SplusH_prev - SminusH_prev (linearity)
                dd = work.tile([P, TC], F32, tag="eek")
                nc.vector.tensor_sub(dd[:], splus[:, 0:TC], smin[:, 0:TC])
                num = nump.tile([P, TC], F32, tag="num")
                nc.vector.scalar_tensor_tensor(
                    num[:], ekv[:], eug[:], dd[:], ALU.mult, ALU.add)
                den = work.tile([P, TC], F32, tag="rp")
                nc.gpsimd.tensor_add(den[:], euk[:], bb[:, 0:TC])
                rcp = work.tile([P, TC], F32, tag="rm")
                nc.vector.reciprocal_approx_fast(rcp[:], den[:])
                nc.vector.tensor_mul(num[:], num[:], rcp[:])
                nc.sync.dma_start(out=wkv_d[c0:c0 + P, t0:t0 + TC], in_=num[:])

                # state outputs: Ln in place over the scan buffers, then DMA.
                # Deferred until the next chunk's carry copies have read the
                # last linear column (emit_lns below runs one chunk behind).
                prev = (splus, smin, bb)
                pending.append((t0, splus, smin, bb))
                if len(pending) > 1:
                    emit_lns(*pending.pop(0))

            for args in pending:
                emit_lns(*args)

    nc.compile()
    return nc


_NC_CACHE = {}


def _get_nc():
    if "nc" not in _NC_CACHE:
        _NC_CACHE["nc"] = build_nc()
    return _NC_CACHE["nc"]


def kernel(w, u, k, v, state):
    w = np.ascontiguousarray(w, dtype=np.float32)
    u = np.ascontiguousarray(u, dtype=np.float32)
    k = np.asarray(k, dtype=np.float32)
    v = np.asarray(v, dtype=np.float32)
    state = np.ascontiguousarray(state, dtype=np.float32)

    nc = _get_nc()
    in_maps = [
        {"k": np.ascontiguousarray(k[b].T), "v": np.ascontiguousarray(v[b].T),
         "w": w, "u": u, "state": state[b]}
        for b in range(B)
    ]
    res = run_bass_kernel_spmd(nc, in_maps, core_ids=list(range(B)))
    wkv = np.stack([res.results[b]["wkv"].T for b in range(B)], axis=0)
    state_out = np.stack(
        [res.results[b]["state_out"].transpose(0, 2, 1) for b in range(B)],
        axis=0)
    return np.ascontiguousarray(wkv), np.ascontiguousarray(state_out)


# revision 16
# speedup vs baseline: 1.0223x; 1.0223x over previous
# RWKV WKV log-space recurrence on 8 TRN2 NeuronCores.
#
# Reference recurrence (per (b, d), sequential over t, log-space):
#   lap' = logaddexp(lap - w, k + ln(relu(v)+eps))
#   lam' = logaddexp(lam - w, k + ln(relu(-v)+eps))
#   lb'  = logaddexp(lb - w,  k)
#   wkv  = exp(lae(u+k+ln v_p, lap) - lae(u+k, lb)) - exp(lae(u+k+ln v_m, lam) - lae(u+k, lb))
#
# Because w >= 0.05 > 0 the recurrence is a stable linear scan in exp space:
#   Ap' = e^-w Ap + e^k (relu(v)+eps),  Am' = e^-w Am + e^k (relu(-v)+eps),
#   B'  = e^-w B  + e^k,   wkv_t = (e^u e^k v + Ap - Am) / (e^u e^k + B).
# Scans are linear, so we scan sum/difference combinations directly:
#   Splus  : b_t = 2 eps e^k + 2 relu(+e^k v),  init 2 Ap0   -> lap = ln(.5 Splus)
#   Sminus : b_t = 2 eps e^k + 2 relu(-e^k v),  init 2 Am0   -> lam = ln(.5 Sminus)
#   Adif   : b_t = e^k v,                       init Ap0-Am0 -> wkv numerator
#   Bb     : b_t = e^k,                         init B0      -> lb, denominator
# (|x|+x = 2 relu(x) removes the Abs; Splus/Sminus replace Asum+-Adif.)
# Each scan is one DVE TensorTensorScanArith: state = d*state + b_t, chained
# across time chunks via the chunk's column-0 carry copy.
#
# Sharding: one batch element per core (B=8). Layout: channels on the 128
# partitions (8 groups of 128), time chunks of TC along the free dim. All
# DRAM I/O is channel-major ([D, T]); the host transposes in kernel() so
# every DMA descriptor is a contiguous 4KB row at full HBM rate.
import numpy as np
from contextlib import ExitStack

import concourse.bacc as bacc
import concourse.bass as bass
import concourse.hw_specs as hw_specs
import concourse.mybir as mybir
import concourse.tile as tile
from concourse.bass_utils import run_bass_kernel_spmd

F32 = mybir.dt.float32
AF = mybir.ActivationFunctionType
ALU = mybir.AluOpType

B, T, D = 8, 2048, 1024
P = 128
EPS = 1e-4

_ACT_TABLES_PATCHED = False


def _patch_act_tables():
    """Make the act-table-load pass always pick `natural_log_exp_and_others`
    (contains Exp+Ln+Relu) instead of thrashing between the exp-only and
    ln-only sets. We strip our functions from every set that precedes it in
    the list; positions (= act_func_set_ids) are unchanged, so walrus's
    remap stays valid."""
    global _ACT_TABLES_PATCHED
    if _ACT_TABLES_PATCHED:
        return
    _ACT_TABLES_PATCHED = True
    orig = hw_specs.get_activation_tables
    ours = {AF.Exp, AF.Ln, AF.Relu, AF.Abs}

    def patched(module_arch):
        tabs = orig(module_arch)
        out = {}
        seen_combined = False
        for name, s in tabs.items():
            if name == "natural_log_exp_and_others":
                seen_combined = True
                out[name] = s
            elif not seen_combined:
                out[name] = s - ours
            else:
                out[name] = s
        return out

    patched.__wrapped__ = orig
    hw_specs.get_activation_tables = patched
    bacc.get_activation_tables = patched


def build_nc(T=T, D=D, TC=1024, n_cores=8):
    """Per-core Bass graph (same NEFF on all cores). I/O is channel-major."""
    _patch_act_tables()
    G = D // P
    CI = T // TC
    nc = bacc.Bacc("TRN2", target_bir_lowering=False, debug=False,
                   num_devices=n_cores)
    k_d = nc.dram_tensor("k", [D, T], F32, kind="ExternalInput").ap()
    v_d = nc.dram_tensor("v", [D, T], F32, kind="ExternalInput").ap()
    w_d = nc.dram_tensor("w", [D], F32, kind="ExternalInput").ap()
    u_d = nc.dram_tensor("u", [D], F32, kind="ExternalInput").ap()
    st_d = nc.dram_tensor("state", [3, 1, D], F32, kind="ExternalInput").ap()
    wkv_d = nc.dram_tensor("wkv", [D, T], F32, kind="ExternalOutput").ap()
    sto_d = nc.dram_tensor("state_out", [3, D, T + 1], F32,
                           kind="ExternalOutput").ap()

    with tile.TileContext(nc) as tc, ExitStack() as ctx:
        const = ctx.enter_context(tc.tile_pool(name="const", bufs=2))
        io = ctx.enter_context(tc.tile_pool(name="io", bufs=3))     # kt, vt
        hot = ctx.enter_context(tc.tile_pool(name="hot", bufs=3))   # ek, ekv
        work = ctx.enter_context(tc.tile_pool(name="work", bufs=3))
        nump = ctx.enter_context(tc.tile_pool(name="nump", bufs=3))  # num/wkv
        scanp = ctx.enter_context(tc.tile_pool(name="scan", bufs=3))

        # t=0 column of the trajectory is the raw input state (12KB, once).
        with nc.allow_non_contiguous_dma(reason="12KB one-time init column"):
            nc.sync.dma_start(out=sto_d[:, :, 0:1],
                              in_=st_d[:, 0, :].unsqueeze(2))

        def make_emit_lns(c0):
            def emit_lns(t0, splus, smin, bb):
                nc.scalar.activation(splus[:, 1:TC + 1], splus[:, 1:TC + 1],
                                     AF.Ln)
                nc.sync.dma_start(out=sto_d[0, c0:c0 + P, t0 + 1:t0 + TC + 1],
                                  in_=splus[:, 1:TC + 1])
                nc.scalar.activation(smin[:, 1:TC + 1], smin[:, 1:TC + 1],
                                     AF.Ln)
                nc.sync.dma_start(out=sto_d[1, c0:c0 + P, t0 + 1:t0 + TC + 1],
                                  in_=smin[:, 1:TC + 1])
                nc.scalar.activation(bb[:, 1:TC + 1], bb[:, 1:TC + 1], AF.Ln)
                nc.sync.dma_start(out=sto_d[2, c0:c0 + P, t0 + 1:t0 + TC + 1],
                                  in_=bb[:, 1:TC + 1])
            return emit_lns

        for g in range(G):
            c0 = g * P
            emit_lns = make_emit_lns(c0)
            pending = []
            wg = const.tile([P, 1], F32, tag="wg")
            nc.sync.dma_start(out=wg[:], in_=w_d[c0:c0 + P].unsqueeze(1))
            ug = const.tile([P, 1], F32, tag="ug")
            nc.sync.dma_start(out=ug[:], in_=u_d[c0:c0 + P].unsqueeze(1))
            stg = const.tile([P, 3], F32, tag="stg")
            nc.sync.dma_start(out=stg[:], in_=st_d[:, 0, c0:c0 + P].transpose([1, 0]))

            dg = const.tile([P, 1], F32, tag="dg")
            nc.scalar.activation(dg[:], wg[:], AF.Exp, scale=-1.0)
            eug = const.tile([P, 1], F32, tag="eug")
            nc.scalar.activation(eug[:], ug[:], AF.Exp)
            est = const.tile([P, 3], F32, tag="est")
            nc.scalar.activation(est[:], stg[:], AF.Exp)
            i_plus = est[:, 0:1]
            i_minus = est[:, 1:2]
            i_b = est[:, 2:3]

            prev = None
            for ci in range(CI):
                t0 = ci * TC
                kt = io.tile([P, TC], F32, tag="kt")
                nc.sync.dma_start(out=kt[:], in_=k_d[c0:c0 + P, t0:t0 + TC])
                vt = io.tile([P, TC], F32, tag="vt")
                nc.sync.dma_start(out=vt[:], in_=v_d[c0:c0 + P, t0:t0 + TC])

                ek = hot.tile([P, TC], F32, tag="ek")
                nc.scalar.activation(ek[:], kt[:], AF.Exp)
                ekv = hot.tile([P, TC], F32, tag="ekv")
                nc.gpsimd.tensor_mul(ekv[:], ek[:], vt[:])
                rp = work.tile([P, TC], F32, tag="rp")
                nc.scalar.activation(rp[:], ekv[:], AF.Relu, scale=1.0)
                rm = work.tile([P, TC], F32, tag="rm")
                nc.scalar.activation(rm[:], ekv[:], AF.Relu, scale=-1.0)
                # eek = 2 eps e^k, euk = e^u e^k (ACT Copy with scale)
                eek = work.tile([P, TC], F32, tag="eek")
                nc.scalar.activation(eek[:], ek[:], AF.Copy, scale=1.0 * EPS)
                euk = work.tile([P, TC], F32, tag="euk")
                nc.scalar.activation(euk[:], ek[:], AF.Copy, scale=eug[:])
                sp_in = work.tile([P, TC], F32, tag="sp_in")
                nc.gpsimd.tensor_add(sp_in[:], eek[:], rp[:])
                sm_in = work.tile([P, TC], F32, tag="sm_in")
                nc.gpsimd.tensor_add(sm_in[:], eek[:], rm[:])

                splus = scanp.tile([P, TC + 1], F32, tag="splus")
                smin = scanp.tile([P, TC + 1], F32, tag="smin")
                bb = scanp.tile([P, TC + 1], F32, tag="bb")
                # column 0 = carry in (raw linear value, copied before the
                # previous chunk's in-place Ln clobbers it)
                if prev is None:
                    nc.vector.tensor_copy(splus[:, 0:1], i_plus)
                    nc.vector.tensor_copy(smin[:, 0:1], i_minus)
                    nc.vector.tensor_copy(bb[:, 0:1], i_b)
                else:
                    nc.vector.tensor_copy(splus[:, 0:1], prev[0][:, TC:TC + 1])
                    nc.vector.tensor_copy(smin[:, 0:1], prev[1][:, TC:TC + 1])
                    nc.vector.tensor_copy(bb[:, 0:1], prev[2][:, TC:TC + 1])
                dbc = dg[:].broadcast_to([P, TC])
                nc.vector.tensor_tensor_scan(
                    splus[:, 1:TC + 1], dbc, sp_in[:], splus[:, 0:1],
                    ALU.mult, ALU.add)
                nc.vector.tensor_tensor_scan(
                    smin[:, 1:TC + 1], dbc, sm_in[:], smin[:, 0:1],
                    ALU.mult, ALU.add)
                nc.vector.tensor_tensor_scan(
                    bb[:, 1:TC + 1], dbc, ek[:], bb[:, 0:1],
                    ALU.mult, ALU.add)

                # Adif_prev = SplusH_prev - SminusH_prev (linearity)
                dd = work.tile([P, TC], F32, tag="dd")
                nc.vector.tensor_sub(dd[:], splus[:, 0:TC], smin[:, 0:TC])
                num = nump.tile([P, TC], F32, tag="num")
                nc.vector.scalar_tensor_tensor(
                    num[:], ekv[:], eug[:], dd[:], ALU.mult, ALU.add)
                den = work.tile([P, TC], F32, tag="den")
                nc.gpsimd.tensor_add(den[:], euk[:], bb[:, 0:TC])
                rcp = work.tile([P, TC], F32, tag="rcp")
                nc.vector.reciprocal_approx_fast(rcp[:], den[:])
                nc.vector.tensor_mul(num[:], num[:], rcp[:])
                nc.sync.dma_start(out=wkv_d[c0:c0 + P, t0:t0 + TC], in_=num[:])

                # state outputs: Ln in place over the scan buffers, then DMA.
                # Deferred until the next chunk's carry copies have read the
                # last linear column (emit_lns below runs one chunk behind).
                prev = (splus, smin, bb)
                pending.append((t0, splus, smin, bb))
                if len(pending) > 1:
                    emit_lns(*pending.pop(0))

            for args in pending:
                emit_lns(*args)

    nc.compile()
    return nc


_NC_CACHE = {}


def _get_nc():
    if "nc" not in _NC_CACHE:
        _NC_CACHE["nc"] = build_nc()
    return _NC_CACHE["nc"]


def kernel(w, u, k, v, state):
    w = np.ascontiguousarray(w, dtype=np.float32)
    u = np.ascontiguousarray(u, dtype=np.float32)
    k = np.asarray(k, dtype=np.float32)
    v = np.asarray(v, dtype=np.float32)
    state = np.ascontiguousarray(state, dtype=np.float32)

    nc = _get_nc()
    in_maps = [
        {"k": np.ascontiguousarray(k[b].T), "v": np.ascontiguousarray(v[b].T),
         "w": w, "u": u, "state": state[b]}
        for b in range(B)
    ]
    res = run_bass_kernel_spmd(nc, in_maps, core_ids=list(range(B)))
    wkv = np.stack([res.results[b]["wkv"].T for b in range(B)], axis=0)
    state_out = np.stack(
        [res.results[b]["state_out"].transpose(0, 2, 1) for b in range(B)],
        axis=0)
    return np.ascontiguousarray(wkv), np.ascontiguousarray(state_out)
